# revision 1
# baseline (speedup 1.0000x reference)
"""Trainium2 Bass kernel for 16-head causal MHA (B=2, T=2048, C=1024, H=16, D=64).

Sharding: 8 cores = 2 batch groups x 4 head groups (4 heads each).
Each core computes, for its batch b and heads hg*4..hg*4+3:
  Q^T,K^T = projections kept transposed [dims, tokens] (fp32r matmuls)
  V       = projection transposed back to [tokens, dims] via PE transpose,
            augmented with a ones column per head (denominator trick)
  S^T     = K Q^T per (ts-tile, tq-chunk), causal-masked on the diagonal
            128-block only (fully-masked columns skipped), exp'd (scale
            folded into Wq on host)
  O^T_aug = V_aug^T P^T accumulated over ts tiles; row 64 is the softmax
            denominator; normalized via GPSIMD partition_broadcast + DVE
  Y_part  = O^T.T @ Wo_slice^T, interleaved per chunk  [2048, 1024]
Host sums the 4 head-group partials per batch and adds bo.
"""

import sys

sys.path.insert(0, "/opt/trn_rl_repo")

import numpy as np

import concourse.bass as bass
from concourse import bacc
import concourse.mybir as mybir
from concourse.tile import TileContext
from concourse.bass_utils import run_bass_kernel_spmd
from concourse.masks import make_identity

F32 = mybir.dt.float32
F32R = mybir.dt.float32r
EXP = mybir.ActivationFunctionType.Exp

B, T, C, H, D = 2, 2048, 1024, 16, 64
NHPC = 4          # heads per core
DH = NHPC * D     # 256 head dims per core
P = 128           # partitions
CH = 512          # token chunk (matmul moving dim)
NCHUNK = T // CH  # 4
NTT = T // P      # 16 token tiles
NCT = C // P      # 8 contraction tiles over C
NEG = -1.0e10


def build_nc(loop_reps=None, stages=3, no_mask=False, no_norm=False,
             interleave_out=True, skip_v=False, skip_proj_copies=False):
    nc = bacc.Bacc()
    xT_d = nc.declare_dram_parameter("xT", [C, T], F32R, isOutput=False)
    wqkv_d = nc.declare_dram_parameter("Wqkv", [C, 3 * DH], F32R, isOutput=False)
    wot_d = nc.declare_dram_parameter("WoT", [DH, C], F32R, isOutput=False)
    y_d = nc.declare_dram_parameter("Y", [T, C], F32, isOutput=True)

    xT = xT_d[:, :]
    wqkv = wqkv_d[:, :]
    wot = wot_d[:, :]
    y = y_d[:, :]

    with TileContext(nc) as tc:
        with (
            tc.tile_pool(name="const", bufs=1) as const,
            tc.tile_pool(name="persist", bufs=1) as persist,
        ):
            # ---- constants ----
            ones_f32 = const.tile([P, 1], F32)
            nc.gpsimd.memset(ones_f32[:], 1.0)
            ones_row = const.tile([1, D], F32)
            nc.gpsimd.memset(ones_row[:], 1.0)
            ones_col = const.tile([1, D], F32R)
            nc.vector.tensor_copy(ones_col[:], ones_row[:])
            # triangular mask for the diagonal 128x128 block (both halves):
            # mask[r, (half, j)] = 0 if r <= j else -1e10
            mask128 = const.tile([P, 2, P], F32, name="mask128")
            nc.gpsimd.memset(mask128[:], 0.0)
            nc.gpsimd.affine_select(
                out=mask128[:],
                in_=mask128[:],
                compare_op=mybir.AluOpType.is_ge,
                fill=NEG,
                base=0,
                pattern=[[0, 2], [1, P]],
                channel_multiplier=-1,
            )

            # ---- persistent tensors ----
            wq_t = [persist.tile([P, 3 * DH], F32R, name=f"wqkv{c}")
                    for c in range(NCT)]
            wot_t = [persist.tile([P, C], F32R, name=f"wot{k}") for k in range(2)]
            # Q^T/K^T [dims, tokens]; pair p holds heads (2p, 2p+1)
            qt_t = [persist.tile([P, T], F32R, name=f"qt{p}") for p in range(2)]
            kt_t = [persist.tile([P, T], F32R, name=f"kt{p}") for p in range(2)]
            # V augmented with a ones column per head: [tokens, 4*65]
            vaug_t = [persist.tile([P, NHPC * (D + 1)], F32R, name=f"vaug{t}")
                      for t in range(NTT)]
            for t in range(NTT):
                for h in range(NHPC):
                    col = h * (D + 1) + D
                    nc.vector.tensor_copy(vaug_t[t][:, col : col + 1], ones_f32[:])
            # normalized O^T [dims, tokens]
            ot_t = [persist.tile([P, T], F32R, name=f"ot{p}") for p in range(2)]

            def emit_weight_dmas():
                for c in range(NCT):
                    nc.sync.dma_start(wq_t[c][:], wqkv[c * P : (c + 1) * P, :])
                for k in range(2):
                    nc.sync.dma_start(wot_t[k][:], wot[k * P : (k + 1) * P, :])

            def emit_stage1():
                with (
                    tc.tile_pool(name="xt", bufs=24) as xt_pool,
                    tc.tile_pool(name="psproj", bufs=4, space="PSUM") as ps_proj,
                ):
                    for n in range(NCHUNK):
                        csl = slice(n * CH, (n + 1) * CH)
                        xts = []
                        for c in range(NCT):
                            xtile = xt_pool.tile([P, CH], F32R, tag="xt",
                                                 name=f"xt{n}_{c}")
                            nc.sync.dma_start(xtile[:], xT[c * P : (c + 1) * P, csl])
                            xts.append(xtile)
                        if n == 0:
                            # weights go to the DMA queues after chunk-0 x tiles
                            emit_weight_dmas()
                        # Q^T/K^T: W stationary, x^T moving -> [dims, tokens]
                        for m in range(4):
                            ps = ps_proj.tile([P, CH], F32, tag="ps",
                                              name=f"ps{n}_{m}")
                            for c in range(NCT):
                                nc.tensor.matmul(
                                    ps[:],
                                    lhsT=wq_t[c][:, m * P : (m + 1) * P],
                                    rhs=xts[c][:],
                                    start=(c == 0),
                                    stop=(c == NCT - 1),
                                )
                            if skip_proj_copies:
                                continue
                            if m < 2:
                                nc.vector.tensor_copy(qt_t[m][:, csl], ps[:])
                            else:
                                nc.vector.tensor_copy(kt_t[m - 2][:, csl], ps[:])
                        if skip_v:
                            continue
                        # V natural: x^T tile stationary, Wv moving -> [tokens, vdims]
                        for j in range(4):
                            vp = ps_proj.tile([P, DH], F32, tag="vp",
                                              name=f"vp{n}_{j}")
                            for c in range(NCT):
                                nc.tensor.matmul(
                                    vp[:],
                                    lhsT=xts[c][:, j * P : (j + 1) * P],
                                    rhs=wq_t[c][:, 2 * DH : 3 * DH],
                                    start=(c == 0),
                                    stop=(c == NCT - 1),
                                )
                            if skip_proj_copies:
                                continue
                            # scatter vdims into the per-head 65-col layout
                            va = vaug_t[4 * n + j]
                            for h in range(NHPC):
                                nc.vector.tensor_copy(
                                    va[:, h * (D + 1) : h * (D + 1) + D],
                                    vp[:, h * D : (h + 1) * D])

            def emit_stage23():
                with (
                    tc.tile_pool(name="pt", bufs=4) as pt_pool,
                    tc.tile_pool(name="small", bufs=4) as small_pool,
                    tc.tile_pool(name="ysb", bufs=3) as y_pool,
                    tc.tile_pool(name="psst", bufs=2, space="PSUM") as ps_st,
                    tc.tile_pool(name="psot", bufs=2, space="PSUM") as ps_ot,
                    tc.tile_pool(name="psy", bufs=2, space="PSUM") as ps_y,
                ):
                    pending_out = []

                    def emit_outproj_tile(tt):
                        tsl = slice(tt * P, (tt + 1) * P)
                        for nn in range(2):
                            nsl = slice(nn * CH, (nn + 1) * CH)
                            yp = ps_y.tile([P, CH], F32, tag="y",
                                           name=f"y{tt}_{nn}")
                            for k in range(2):
                                nc.tensor.matmul(
                                    yp[:],
                                    lhsT=ot_t[k][:, tsl],
                                    rhs=wot_t[k][:, nsl],
                                    start=(k == 0),
                                    stop=(k == 1),
                                )
                            ysb = y_pool.tile([P, CH], F32, tag="ysb",
                                              name=f"ysb{tt}_{nn}")
                            if (tt + nn) % 2 == 0:
                                nc.vector.tensor_copy(ysb[:], yp[:])
                            else:
                                nc.scalar.copy(ysb[:], yp[:])
                            nc.sync.dma_start(y[tsl, nsl], ysb[:])

                    for cq in range(NCHUNK):
                        qsl = slice(cq * CH, (cq + 1) * CH)
                        nts = 4 * cq + 4

                        def emit_st(t, cq=cq, p=None):
                            st = ps_st.tile([P, 2, CH], F32, tag="st",
                                            name=f"st{cq}_{p}_{t}")
                            tsl = slice(t * P, (t + 1) * P)
                            js = max(0, (t - 4 * cq) * P)
                            qs = slice(cq * CH + js, (cq + 1) * CH)
                            for hh in range(2):
                                nc.tensor.matmul(
                                    st[:, hh, js:],
                                    lhsT=kt_t[p][hh * D : (hh + 1) * D, tsl],
                                    rhs=qt_t[p][hh * D : (hh + 1) * D, qs],
                                    start=True,
                                    stop=True,
                                )
                            if t >= 4 * cq and not no_mask:
                                nc.vector.tensor_add(
                                    st[:, :, js : js + P],
                                    st[:, :, js : js + P],
                                    mask128[:],
                                )
                            return st, js

                        for p in range(2):
                            ots = [
                                ps_ot.tile([D + 1, CH], F32, tag="ot",
                                           name=f"ot{cq}_{p}_{hh}")
                                for hh in range(2)
                            ]
                            sts = {0: emit_st(0, p=p)}
                            for t in range(nts):
                                # pipeline: next tile's scores go ahead of AV
                                if t + 1 < nts:
                                    sts[t + 1] = emit_st(t + 1, p=p)
                                st, js = sts.pop(t)
                                pt = pt_pool.tile([P, 2, CH], F32R, tag="pt",
                                                  name=f"pt{cq}_{p}_{t}")
                                nc.scalar.activation(pt[:, :, js:], st[:, :, js:],
                                                     EXP)
                                for hh in range(2):
                                    h = 2 * p + hh
                                    nc.tensor.matmul(
                                        ots[hh][:, js:],
                                        lhsT=vaug_t[t][:, h * (D + 1)
                                                       : (h + 1) * (D + 1)],
                                        rhs=pt[:, hh, js:],
                                        start=(t == 0),
                                        stop=(t == nts - 1),
                                    )
                                # spread previous chunk's out-projection
                                if pending_out and t >= 1:
                                    emit_outproj_tile(pending_out.pop(0))
                            for hh in range(2):
                                ot = ots[hh]
                                if no_norm:
                                    with nc.allow_low_precision("timing variant"):
                                        nc.vector.tensor_copy(
                                            ot_t[p][hh * D : (hh + 1) * D, qsl],
                                            ot[0:D, :])
                                    continue
                                # bounce to SBUF (frees the PSUM slot quickly)
                                otu = small_pool.tile([D + 1, CH], F32, tag="otu",
                                                      name=f"otu{cq}_{p}_{hh}")
                                nc.vector.tensor_copy(otu[:], ot[:])
                                # 1/denom at partition 0, then GPSIMD broadcast
                                recip = small_pool.tile([1, CH], F32, tag="rc",
                                                        name=f"rc{cq}_{p}_{hh}")
                                nc.vector.reciprocal(recip[:], otu[D : D + 1, :])
                                den = small_pool.tile([D, CH], F32, tag="den",
                                                      name=f"dn{cq}_{p}_{hh}")
                                nc.gpsimd.partition_broadcast(den[:], recip[:])
                                with nc.allow_low_precision("fp32r store"):
                                    nc.vector.tensor_mul(
                                        ot_t[p][hh * D : (hh + 1) * D, qsl],
                                        otu[0:D, :],
                                        den[:],
                                    )
                        if stages >= 3:
                            pending_out.extend(range(4 * cq, 4 * cq + 4))
                    while pending_out:
                        emit_outproj_tile(pending_out.pop(0))

            def emit_dbg_outputs():
                if stages == 1:
                    dbg_srcs = ((wq_t[0], wq_t[1], wq_t[2], wq_t[3])
                                if skip_proj_copies else
                                (qt_t[0], qt_t[1], kt_t[0], kt_t[1]))
                    for i, src_t in enumerate(dbg_srcs):
                        w = min(C, src_t.shape[1])
                        nc.sync.dma_start(y[i * P : (i + 1) * P, 0:w],
                                          src_t[:, 0:w].bitcast(F32))
                    with tc.tile_pool(name="dbg", bufs=2) as dbgp:
                        for tt in range(4):
                            db = dbgp.tile([P, C], F32, tag="db", name=f"db{tt}")
                            nc.gpsimd.memset(db[:], 0.0)
                            for j in range(4):
                                nc.vector.tensor_copy(
                                    db[:, j * 256 : j * 256 + 256],
                                    vaug_t[4 * tt + j][:, 0:256].bitcast(F32))
                            nc.sync.dma_start(y[(4 + tt) * P : (5 + tt) * P, :],
                                              db[:])
                elif stages == 2:
                    for i, src_t in enumerate((ot_t[0], ot_t[1])):
                        nc.sync.dma_start(y[i * P : (i + 1) * P, :],
                                          src_t[:, 0:C].bitcast(F32))

            def emit_body():
                emit_stage1()
                if stages >= 2:
                    emit_stage23()
                emit_dbg_outputs()

            if loop_reps is None:
                emit_body()
            else:
                with tc.For_i(0, loop_reps, 1):
                    emit_body()

    nc.finalize()
    return nc


_NC_CACHE = None


def get_nc():
    global _NC_CACHE
    if _NC_CACHE is None:
        _NC_CACHE = build_nc()
    return _NC_CACHE


def make_in_maps(x, Wq, Wk, Wv, Wo):
    scale = 1.0 / np.sqrt(np.float32(C))
    in_maps = []
    for core in range(8):
        b, hg = core // 4, core % 4
        hsl = slice(hg * NHPC, (hg + 1) * NHPC)
        xT = np.ascontiguousarray(x[b].T)
        wq = (Wq[hsl] * scale).transpose(1, 0, 2).reshape(C, DH)
        wk = Wk[hsl].transpose(1, 0, 2).reshape(C, DH)
        wv = Wv[hsl].transpose(1, 0, 2).reshape(C, DH)
        wqkv = np.ascontiguousarray(
            np.concatenate([wq, wk, wv], axis=1, dtype=np.float32))
        wot = np.ascontiguousarray(Wo[:, hg * DH : (hg + 1) * DH].T)
        in_maps.append({
            "xT": xT.astype(np.float32, copy=False),
            "Wqkv": wqkv,
            "WoT": wot.astype(np.float32, copy=False),
        })
    return in_maps


def gather(results, bo):
    out = np.zeros((B, T, C), dtype=np.float32)
    for core in range(8):
        out[core // 4] += results[core]["Y"]
    out += bo.astype(np.float32)
    return out


def kernel(x, Wq, Wk, Wv, Wo, bo, **run_kwargs):
    x = np.asarray(x, dtype=np.float32)
    Wq = np.asarray(Wq, dtype=np.float32)
    Wk = np.asarray(Wk, dtype=np.float32)
    Wv = np.asarray(Wv, dtype=np.float32)
    Wo = np.asarray(Wo, dtype=np.float32)
    bo = np.asarray(bo, dtype=np.float32)
    nc = get_nc()
    in_maps = make_in_maps(x, Wq, Wk, Wv, Wo)
    res = run_bass_kernel_spmd(nc, in_maps, core_ids=list(range(8)), **run_kwargs)
    out = gather(res.results, bo)
    if run_kwargs:
        return out, res
    return out



# revision 2
# speedup vs baseline: 23746.7073x; 23746.7073x over previous
"""Trainium2 Bass kernel for 16-head causal MHA (B=2, T=2048, C=1024, H=16, D=64).

Sharding: 8 cores = 2 batch groups x 4 head groups (4 heads each).
All matmuls run in bf16 (inputs pre-cast on host; fp32 PSUM accumulate).
Each core computes, for its batch b and heads hg*4..hg*4+3:
  Q^T,K^T = projections kept transposed [dims, tokens]
  V       = projection in [tokens, dims] layout, augmented with a ones
            column per head (softmax-denominator trick)
  S^T     = K Q^T per (ts-tile, tq-chunk), causal-masked on the diagonal
            128-blocks only (fully-masked tiles skipped), exp'd on ACT
            (scale folded into Wq on host)
  O^T_aug = V_aug^T P^T accumulated over ts tiles; row 64 is the softmax
            denominator; normalized via reciprocal_approx_fast +
            gpsimd partition_broadcast + DVE multiply from PSUM
  Y_part  = O^T.T @ Wo_slice^T, interleaved per chunk  [2048, 1024]
Host sums the 4 head-group partials per batch and adds bo.
"""

import sys

sys.path.insert(0, "/opt/trn_rl_repo")

import numpy as np
import ml_dtypes

import concourse.bass as bass
from concourse import bacc
import concourse.mybir as mybir
from concourse.tile import TileContext
from concourse.bass_utils import run_bass_kernel_spmd

F32 = mybir.dt.float32
BF16 = mybir.dt.bfloat16
EXP = mybir.ActivationFunctionType.Exp

B, T, C, H, D = 2, 2048, 1024, 16, 64
NHPC = 4          # heads per core
DH = NHPC * D     # 256 head dims per core
P = 128           # partitions
CH = 512          # token chunk (matmul moving dim)
NCHUNK = T // CH  # 4
NTT = T // P      # 16 token tiles
NCT = C // P      # 8 contraction tiles over C
NEG = -30000.0    # masked-score fill; exp(-30000) == 0 in fp32


def build_nc():
    nc = bacc.Bacc()
    xT_d = nc.declare_dram_parameter("xT", [C, T], BF16, isOutput=False)
    wqkv_d = nc.declare_dram_parameter("Wqkv", [C, 3 * DH], BF16, isOutput=False)
    wot_d = nc.declare_dram_parameter("WoT", [DH, C], BF16, isOutput=False)
    y_d = nc.declare_dram_parameter("Y", [T, C], F32, isOutput=True)

    xT = xT_d[:, :]
    wqkv = wqkv_d[:, :]
    wot = wot_d[:, :]
    y = y_d[:, :]

    with TileContext(nc) as tc:
        with (
            tc.tile_pool(name="const", bufs=1) as const,
            tc.tile_pool(name="persist", bufs=1) as persist,
        ):
            # ---- constants ----
            # triangular mask for the diagonal 128x128 block (both heads):
            # mask[r, (head, j)] = 0 if r <= j else NEG
            mask128 = const.tile([P, 2, P], F32, name="mask128")
            nc.gpsimd.memset(mask128[:], 0.0)
            nc.gpsimd.affine_select(
                out=mask128[:],
                in_=mask128[:],
                compare_op=mybir.AluOpType.is_ge,
                fill=NEG,
                base=0,
                pattern=[[0, 2], [1, P]],
                channel_multiplier=-1,
            )

            # ---- persistent tensors ----
            wq_t = [persist.tile([P, 3 * DH], BF16, name=f"wqkv{c}")
                    for c in range(NCT)]
            wot_t = [persist.tile([P, C], BF16, name=f"wot{k}") for k in range(2)]
            # Q^T/K^T [dims, tokens]; pair p holds heads (2p, 2p+1)
            qt_t = [persist.tile([P, T], BF16, name=f"qt{p}") for p in range(2)]
            kt_t = [persist.tile([P, T], BF16, name=f"kt{p}") for p in range(2)]
            # V augmented with a ones column per head: [tokens, 4, 65]
            vaug_t = [persist.tile([P, NHPC, D + 1], BF16, name=f"vaug{t}")
                      for t in range(NTT)]
            for t in range(NTT):
                nc.gpsimd.memset(vaug_t[t][:, :, D : D + 1], 1.0)
            # normalized O^T [dims, tokens]
            ot_t = [persist.tile([P, T], BF16, name=f"ot{p}") for p in range(2)]

            def emit_weight_dmas():
                for c in range(NCT):
                    nc.sync.dma_start(wq_t[c][:], wqkv[c * P : (c + 1) * P, :])
                for k in range(2):
                    nc.sync.dma_start(wot_t[k][:], wot[k * P : (k + 1) * P, :])

            def emit_stage1():
                with (
                    tc.tile_pool(name="xt", bufs=24) as xt_pool,
                    tc.tile_pool(name="psproj", bufs=4, space="PSUM") as ps_proj,
                ):
                    for n in range(NCHUNK):
                        csl = slice(n * CH, (n + 1) * CH)
                        xts = []
                        for c in range(NCT):
                            xtile = xt_pool.tile([P, CH], BF16, tag="xt",
                                                 name=f"xt{n}_{c}")
                            nc.sync.dma_start(xtile[:], xT[c * P : (c + 1) * P, csl])
                            xts.append(xtile)
                        if n == 0:
                            # weights go to the DMA queues after chunk-0 x tiles
                            emit_weight_dmas()
                        # Q^T/K^T: W stationary, x^T moving -> [dims, tokens]
                        for m in range(4):
                            ps = ps_proj.tile([P, CH], F32, tag="ps",
                                              name=f"ps{n}_{m}")
                            for c in range(NCT):
                                nc.tensor.matmul(
                                    ps[:],
                                    lhsT=wq_t[c][:, m * P : (m + 1) * P],
                                    rhs=xts[c][:],
                                    start=(c == 0),
                                    stop=(c == NCT - 1),
                                )
                            with nc.allow_low_precision("bf16 store"):
                                if m < 2:
                                    nc.vector.tensor_copy(qt_t[m][:, csl], ps[:])
                                else:
                                    nc.vector.tensor_copy(kt_t[m - 2][:, csl],
                                                          ps[:])
                        # V natural: x^T tile stationary, Wv moving -> [tokens, vdims]
                        for j in range(4):
                            vp = ps_proj.tile([P, DH], F32, tag="vp",
                                              name=f"vp{n}_{j}")
                            for c in range(NCT):
                                nc.tensor.matmul(
                                    vp[:],
                                    lhsT=xts[c][:, j * P : (j + 1) * P],
                                    rhs=wq_t[c][:, 2 * DH : 3 * DH],
                                    start=(c == 0),
                                    stop=(c == NCT - 1),
                                )
                            # one strided copy scatters vdims into the
                            # per-head 65-col layout
                            va = vaug_t[4 * n + j]
                            with nc.allow_low_precision("bf16 store"):
                                nc.vector.tensor_copy(
                                    va[:, :, 0:D],
                                    vp[:].rearrange("p (h d) -> p h d", h=NHPC))

            def emit_stage23():
                with (
                    tc.tile_pool(name="pt", bufs=4) as pt_pool,
                    tc.tile_pool(name="small", bufs=4) as small_pool,
                    tc.tile_pool(name="ysb", bufs=3) as y_pool,
                    tc.tile_pool(name="psst", bufs=2, space="PSUM") as ps_st,
                    tc.tile_pool(name="psot", bufs=2, space="PSUM") as ps_ot,
                    tc.tile_pool(name="psy", bufs=2, space="PSUM") as ps_y,
                ):
                    pending_out = []

                    def emit_outproj_tile(tt):
                        tsl = slice(tt * P, (tt + 1) * P)
                        for nn in range(2):
                            nsl = slice(nn * CH, (nn + 1) * CH)
                            yp = ps_y.tile([P, CH], F32, tag="y",
                                           name=f"y{tt}_{nn}")
                            for k in range(2):
                                nc.tensor.matmul(
                                    yp[:],
                                    lhsT=ot_t[k][:, tsl],
                                    rhs=wot_t[k][:, nsl],
                                    start=(k == 0),
                                    stop=(k == 1),
                                )
                            ysb = y_pool.tile([P, CH], F32, tag="ysb",
                                              name=f"ysb{tt}_{nn}")
                            if (tt + nn) % 2 == 0:
                                nc.vector.tensor_copy(ysb[:], yp[:])
                            else:
                                nc.scalar.copy(ysb[:], yp[:])
                            nc.sync.dma_start(y[tsl, nsl], ysb[:])

                    for cq in range(NCHUNK):
                        qsl = slice(cq * CH, (cq + 1) * CH)
                        nts = 4 * cq + 4

                        def emit_st(t, cq=cq, p=None):
                            st = ps_st.tile([P, 2, CH], F32, tag="st",
                                            name=f"st{cq}_{p}_{t}")
                            tsl = slice(t * P, (t + 1) * P)
                            js = max(0, (t - 4 * cq) * P)
                            qs = slice(cq * CH + js, (cq + 1) * CH)
                            for hh in range(2):
                                nc.tensor.matmul(
                                    st[:, hh, js:],
                                    lhsT=kt_t[p][hh * D : (hh + 1) * D, tsl],
                                    rhs=qt_t[p][hh * D : (hh + 1) * D, qs],
                                    start=True,
                                    stop=True,
                                )
                            if t >= 4 * cq:
                                nc.vector.tensor_add(
                                    st[:, :, js : js + P],
                                    st[:, :, js : js + P],
                                    mask128[:],
                                )
                            return st, js

                        for p in range(2):
                            ots = [
                                ps_ot.tile([D + 1, CH], F32, tag="ot",
                                           name=f"ot{cq}_{p}_{hh}")
                                for hh in range(2)
                            ]
                            sts = {0: emit_st(0, p=p)}
                            for t in range(nts):
                                # pipeline: next tile's scores go ahead of AV
                                if t + 1 < nts:
                                    sts[t + 1] = emit_st(t + 1, p=p)
                                st, js = sts.pop(t)
                                pt = pt_pool.tile([P, 2, CH], BF16, tag="pt",
                                                  name=f"pt{cq}_{p}_{t}")
                                nc.scalar.activation(pt[:, :, js:], st[:, :, js:],
                                                     EXP)
                                for hh in range(2):
                                    h = 2 * p + hh
                                    nc.tensor.matmul(
                                        ots[hh][:, js:],
                                        lhsT=vaug_t[t][:, h, :],
                                        rhs=pt[:, hh, js:],
                                        start=(t == 0),
                                        stop=(t == nts - 1),
                                    )
                                # spread previous chunk's out-projection
                                if pending_out and t >= 1:
                                    emit_outproj_tile(pending_out.pop(0))
                            # ---- softmax normalization for this (cq, p) ----
                            # denominators live at row D of each ots[hh]
                            den = small_pool.tile([1, 2, CH], F32, tag="den",
                                                  name=f"dn{cq}_{p}")
                            for hh in range(2):
                                nc.scalar.copy(den[:, hh, :],
                                               ots[hh][D : D + 1, :])
                            recd = small_pool.tile([1, 2, CH], F32, tag="rc",
                                                   name=f"rc{cq}_{p}")
                            nc.vector.reciprocal_approx_fast(recd[:], den[:])
                            recb = small_pool.tile([D, 2, CH], F32, tag="rb",
                                                   name=f"rb{cq}_{p}")
                            nc.gpsimd.partition_broadcast(recb[:], recd[:])
                            with nc.allow_low_precision("bf16 store"):
                                for hh in range(2):
                                    nc.vector.tensor_mul(
                                        ot_t[p][hh * D : (hh + 1) * D, qsl],
                                        ots[hh][0:D, :],
                                        recb[:, hh, :],
                                    )
                        pending_out.extend(range(4 * cq, 4 * cq + 4))
                    while pending_out:
                        emit_outproj_tile(pending_out.pop(0))

            emit_stage1()
            emit_stage23()

    nc.finalize()
    return nc


_NC_CACHE = None


def get_nc():
    global _NC_CACHE
    if _NC_CACHE is None:
        _NC_CACHE = build_nc()
    return _NC_CACHE


def make_in_maps(x, Wq, Wk, Wv, Wo):
    scale = 1.0 / np.sqrt(np.float32(C))
    bf = ml_dtypes.bfloat16
    in_maps = []
    for core in range(8):
        b, hg = core // 4, core % 4
        hsl = slice(hg * NHPC, (hg + 1) * NHPC)
        xT = np.ascontiguousarray(x[b].T.astype(bf))
        wq = (Wq[hsl] * scale).transpose(1, 0, 2).reshape(C, DH)
        wk = Wk[hsl].transpose(1, 0, 2).reshape(C, DH)
        wv = Wv[hsl].transpose(1, 0, 2).reshape(C, DH)
        wqkv = np.ascontiguousarray(
            np.concatenate([wq, wk, wv], axis=1).astype(bf))
        wot = np.ascontiguousarray(Wo[:, hg * DH : (hg + 1) * DH].T.astype(bf))
        in_maps.append({
            "xT": xT,
            "Wqkv": wqkv,
            "WoT": wot,
        })
    return in_maps


def gather(results, bo):
    out = np.zeros((B, T, C), dtype=np.float32)
    for core in range(8):
        out[core // 4] += results[core]["Y"]
    out += bo.astype(np.float32)
    return out


def kernel(x, Wq, Wk, Wv, Wo, bo, **run_kwargs):
    x = np.asarray(x, dtype=np.float32)
    Wq = np.asarray(Wq, dtype=np.float32)
    Wk = np.asarray(Wk, dtype=np.float32)
    Wv = np.asarray(Wv, dtype=np.float32)
    Wo = np.asarray(Wo, dtype=np.float32)
    bo = np.asarray(bo, dtype=np.float32)
    nc = get_nc()
    in_maps = make_in_maps(x, Wq, Wk, Wv, Wo)
    res = run_bass_kernel_spmd(nc, in_maps, core_ids=list(range(8)), **run_kwargs)
    out = gather(res.results, bo)
    if run_kwargs:
        return out, res
    return out


# revision 13
# speedup vs baseline: 29443.4236x; 1.2399x over previous
"""Trainium2 Bass kernel for 16-head causal MHA (B=2, T=2048, C=1024, H=16, D=64).

Sharding: 8 cores = 2 batch groups x 4 head groups (4 heads each).
All matmuls run in bf16 (inputs pre-cast on host; fp32 PSUM accumulate).

v3: single fused pipeline keeping the PE dense and the ACT-bound exp stream
overlapped:
  - projection matmul groups for chunk n+1 and out-projection tiles for
    chunk n-1 drain as PE "filler" inside chunk n's attention t-loops
  - causal mask applied on the PE (identity-matmul accumulate of a
    triangular bf16 constant into the score PSUM)
  - AV output PSUM freed immediately via an SBUF bounce copy; softmax
    normalization (reciprocal_approx_fast + gpsimd partition_broadcast +
    DVE multiply) runs decoupled, with the multiplies deferred into the
    next loop so no engine head-of-line blocks on the broadcast
  - PE warm-up matmuls flip the HAM clock gate to 8/8 during the initial
    DMA wait
Host sums the 4 head-group partials per batch and adds bo.
"""

import sys

sys.path.insert(0, "/opt/trn_rl_repo")

import numpy as np
import ml_dtypes

import concourse.bass as bass
from concourse import bacc
import concourse.mybir as mybir
from concourse.tile import TileContext
from concourse.bass_utils import run_bass_kernel_spmd
from concourse.masks import make_identity

F32 = mybir.dt.float32
BF16 = mybir.dt.bfloat16
EXP = mybir.ActivationFunctionType.Exp

B, T, C, H, D = 2, 2048, 1024, 16, 64
NHPC = 4          # heads per core
DH = NHPC * D     # 256 head dims per core
P = 128           # partitions
CH = 512          # token chunk (matmul moving dim)
NCHUNK = T // CH  # 4
NTT = T // P      # 16 token tiles
NCT = C // P      # 8 contraction tiles over C
NEG = -30000.0    # masked-score fill; exp() flushes to 0


import os

_DISABLE = set(os.environ.get("KERNEL_DISABLE", "").split(","))


def build_nc():
    no_warmup = "warmup" in _DISABLE
    no_defer = "defer" in _DISABLE
    no_interleave = "interleave" in _DISABLE
    nc = bacc.Bacc()
    xT_d = nc.declare_dram_parameter("xT", [C, T], BF16, isOutput=False)
    wqkv_d = nc.declare_dram_parameter("Wqkv", [C, 3 * DH], BF16, isOutput=False)
    wot_d = nc.declare_dram_parameter("WoT", [DH, C], BF16, isOutput=False)
    y_d = nc.declare_dram_parameter("Y", [T, C], F32, isOutput=True)

    xT = xT_d[:, :]
    wqkv = wqkv_d[:, :]
    wot = wot_d[:, :]
    y = y_d[:, :]

    with TileContext(nc) as tc:
        with (
            tc.tile_pool(name="const", bufs=1) as const,
            tc.tile_pool(name="persist", bufs=1) as persist,
            tc.tile_pool(name="xt", bufs=24) as xt_pool,
            tc.tile_pool(name="pt", bufs=4) as pt_pool,
            tc.tile_pool(name="small", bufs=4) as small_pool,
            tc.tile_pool(name="ysb", bufs=4) as y_pool,
            tc.tile_pool(name="psproj", bufs=2, space="PSUM") as ps_proj,
            tc.tile_pool(name="psst", bufs=2, space="PSUM") as ps_st,
            tc.tile_pool(name="psot", bufs=2, space="PSUM") as ps_ot,
        ):
            # ---- constants ----
            id_f32 = const.tile([P, P], F32, name="idf")
            make_identity(nc, id_f32[:])
            id128 = const.tile([P, P], BF16, name="id128")
            # strictly-upper-triangular NEG (transposed causal mask):
            # maskT[c, i] = NEG if i > c else 0
            maskT_f32 = const.tile([P, P], F32, name="mtf")
            nc.gpsimd.memset(maskT_f32[:], 0.0)
            nc.gpsimd.affine_select(
                out=maskT_f32[:],
                in_=maskT_f32[:],
                compare_op=mybir.AluOpType.is_ge,
                fill=NEG,
                base=0,
                pattern=[[-1, P]],
                channel_multiplier=1,
            )
            maskT = const.tile([P, P], BF16, name="maskT")
            with nc.allow_low_precision("bf16 consts"):
                nc.vector.tensor_copy(id128[:], id_f32[:])
                nc.vector.tensor_copy(maskT[:], maskT_f32[:])
            # DVE-side causal mask (baseline style):
            # mask128[r, (hh, j)] = 0 if r <= j else NEG
            mask128 = const.tile([P, 2, P], F32, name="mask128")
            nc.gpsimd.memset(mask128[:], 0.0)
            nc.gpsimd.affine_select(
                out=mask128[:],
                in_=mask128[:],
                compare_op=mybir.AluOpType.is_ge,
                fill=NEG,
                base=0,
                pattern=[[0, 2], [1, P]],
                channel_multiplier=-1,
            )

            # ---- persistent tensors ----
            wq_t = [persist.tile([P, 3 * DH], BF16, name=f"wqkv{c}")
                    for c in range(NCT)]
            wot_t = [persist.tile([P, C], BF16, name=f"wot{k}") for k in range(2)]
            # Q^T/K^T [dims, tokens]; pair p holds heads (2p, 2p+1)
            qt_t = [persist.tile([P, T], BF16, name=f"qt{p}") for p in range(2)]
            kt_t = [persist.tile([P, T], BF16, name=f"kt{p}") for p in range(2)]
            # V augmented with a ones column per head: [tokens, 4, 65]
            vaug_t = [persist.tile([P, NHPC, D + 1], BF16, name=f"vaug{t}")
                      for t in range(NTT)]
            for t in range(NTT):
                nc.gpsimd.memset(vaug_t[t][:, :, D : D + 1], 1.0)
            # normalized O^T [dims, tokens]
            ot_t = [persist.tile([P, T], BF16, name=f"ot{p}") for p in range(2)]

            # ---- DMAs ----
            xts = {}  # (chunk, c) -> tile

            def emit_x_dmas(n):
                csl = slice(n * CH, (n + 1) * CH)
                for c in range(NCT):
                    xtile = xt_pool.tile([P, CH], BF16, tag="xt",
                                         name=f"xt{n}_{c}")
                    nc.sync.dma_start(xtile[:], xT[c * P : (c + 1) * P, csl])
                    xts[(n, c)] = xtile

            emit_x_dmas(0)
            for c in range(NCT):
                nc.sync.dma_start(wq_t[c][:], wqkv[c * P : (c + 1) * P, :])
            for k in range(2):
                nc.sync.dma_start(wot_t[k][:], wot[k * P : (k + 1) * P, :])
            emit_x_dmas(1)

            # ---- PE warm-up (runs during the initial DMA wait) ----
            if not no_warmup:
                warm = ps_proj.tile([P, CH], F32, tag="pp", name="warm")
                for _ in range(30):
                    nc.tensor.matmul(warm[:, 0:P], lhsT=id128[:], rhs=maskT[:],
                                     start=True, stop=True)

            # ---- stage-1 projection groups (PE filler units) ----
            def emit_proj_qk(n, m):
                csl = slice(n * CH, (n + 1) * CH)
                ps = ps_proj.tile([P, CH], F32, tag="pp", name=f"ps{n}_{m}")
                for c in range(NCT):
                    nc.tensor.matmul(
                        ps[:],
                        lhsT=wq_t[c][:, m * P : (m + 1) * P],
                        rhs=xts[(n, c)][:],
                        start=(c == 0),
                        stop=(c == NCT - 1),
                    )
                with nc.allow_low_precision("bf16 store"):
                    if m < 2:
                        nc.vector.tensor_copy(qt_t[m][:, csl], ps[:])
                    else:
                        nc.vector.tensor_copy(kt_t[m - 2][:, csl], ps[:])

            def emit_proj_v(n, j):
                vp = ps_proj.tile([P, CH], F32, tag="pp", name=f"vp{n}_{j}")
                for c in range(NCT):
                    nc.tensor.matmul(
                        vp[:, 0:DH],
                        lhsT=xts[(n, c)][:, j * P : (j + 1) * P],
                        rhs=wq_t[c][:, 2 * DH : 3 * DH],
                        start=(c == 0),
                        stop=(c == NCT - 1),
                    )
                va = vaug_t[4 * n + j]
                with nc.allow_low_precision("bf16 store"):
                    nc.vector.tensor_copy(
                        va[:, :, 0:D],
                        vp[:, 0:DH].rearrange("p (h d) -> p h d", h=NHPC))

            def proj_chunk_units(n):
                units = [(lambda m=m, n=n: emit_proj_qk(n, m)) for m in range(4)]
                units += [(lambda j=j, n=n: emit_proj_v(n, j)) for j in range(4)]
                return units

            # chunk 0 runs before the attention loops start
            for u in proj_chunk_units(0):
                u()

            # ---- out-projection tile (PE filler unit) ----
            def emit_outproj_tile(tt):
                tsl = slice(tt * P, (tt + 1) * P)
                for nn in range(2):
                    nsl = slice(nn * CH, (nn + 1) * CH)
                    yp = ps_proj.tile([P, CH], F32, tag="pp",
                                      name=f"y{tt}_{nn}")
                    for k in range(2):
                        nc.tensor.matmul(
                            yp[:],
                            lhsT=ot_t[k][:, tsl],
                            rhs=wot_t[k][:, nsl],
                            start=(k == 0),
                            stop=(k == 1),
                        )
                    ysb = y_pool.tile([P, CH], F32, tag="ysb",
                                      name=f"ysb{tt}_{nn}")
                    nc.vector.tensor_copy(ysb[:], yp[:])
                    nc.sync.dma_start(y[tsl, nsl], ysb[:])

            # ---- fused attention with filler drain ----
            filler = []          # list of (kind, emit_fn)
            pending_mults = []   # deferred normalization multiplies

            def drain(kinds=("proj", "out"), limit=1):
                done = 0
                i = 0
                while i < len(filler) and done < limit:
                    kind, fn = filler[i]
                    if kind in kinds:
                        filler.pop(i)
                        fn()
                        done += 1
                    else:
                        i += 1

            for cq in range(NCHUNK):
                if cq + 1 <= NCHUNK - 1:
                    units = proj_chunk_units(cq + 1)
                    if no_interleave:
                        for u in units:
                            u()
                    else:
                        filler.extend(("proj", u) for u in units)
                if cq + 2 <= NCHUNK - 1:
                    emit_x_dmas(cq + 2)
                qsl = slice(cq * CH, (cq + 1) * CH)
                nts = 4 * cq + 4

                def emit_st(t, cq=cq, p=None):
                    st = ps_st.tile([P, 2, CH], F32, tag="st",
                                    name=f"st{cq}_{p}_{t}")
                    tsl = slice(t * P, (t + 1) * P)
                    js = max(0, (t - 4 * cq) * P)
                    qs = slice(cq * CH + js, (cq + 1) * CH)
                    diag = t >= 4 * cq
                    for hh in range(2):
                        nc.tensor.matmul(
                            st[:, hh, js:],
                            lhsT=kt_t[p][hh * D : (hh + 1) * D, tsl],
                            rhs=qt_t[p][hh * D : (hh + 1) * D, qs],
                            start=True,
                            stop=True,
                        )
                    if diag:
                        nc.vector.tensor_add(
                            st[:, :, js : js + P],
                            st[:, :, js : js + P],
                            mask128[:],
                        )
                    return st, js

                for p in range(2):
                    ots = [
                        ps_ot.tile([D + 1, CH], F32, tag="ot",
                                   name=f"ot{cq}_{p}_{hh}")
                        for hh in range(2)
                    ]
                    sts = {0: emit_st(0, p=p)}
                    for t in range(nts):
                        if t == 2 and pending_mults:
                            for fn in pending_mults:
                                fn()
                            pending_mults.clear()
                            if cq > 0 and p == 0:
                                # previous chunk fully normalized now
                                for tt in range(4 * (cq - 1), 4 * cq):
                                    filler.append(
                                        ("out",
                                         lambda tt=tt: emit_outproj_tile(tt)))
                        # pipeline: next tile's scores go ahead of AV
                        if t + 1 < nts:
                            sts[t + 1] = emit_st(t + 1, p=p)
                        st, js = sts.pop(t)
                        pt = pt_pool.tile([P, 2, CH], BF16, tag="pt",
                                          name=f"pt{cq}_{p}_{t}")
                        nc.scalar.activation(pt[:, :, js:], st[:, :, js:], EXP)
                        for hh in range(2):
                            h = 2 * p + hh
                            nc.tensor.matmul(
                                ots[hh][:, js:],
                                lhsT=vaug_t[t][:, h, :],
                                rhs=pt[:, hh, js:],
                                start=(t == 0),
                                stop=(t == nts - 1),
                            )
                        drain(limit=2 if cq == 0 else 1)
                    # ---- decoupled softmax normalization for (cq, p) ----
                    # bounce AV PSUM to SBUF right away (frees the ring)
                    otu = [small_pool.tile([D + 1, CH], F32, tag=f"otu{hh}",
                                           name=f"otu{cq}_{p}_{hh}")
                           for hh in range(2)]
                    for hh in range(2):
                        nc.vector.tensor_copy(otu[hh][:], ots[hh][:])
                    # denominator row to partition 0 via ACT (cross-partition
                    # copies are only safe on the scalar engine)
                    den = small_pool.tile([1, 2, CH], F32, tag="dn",
                                          name=f"dn{cq}_{p}")
                    for hh in range(2):
                        nc.scalar.copy(den[:, hh, :], ots[hh][D : D + 1, :])
                    recd = small_pool.tile([1, 2, CH], F32, tag="rc",
                                           name=f"rc{cq}_{p}")
                    nc.vector.reciprocal_approx_fast(recd[:], den[:])
                    recb = small_pool.tile([D, 2, CH], F32, tag="rb",
                                           name=f"rb{cq}_{p}")
                    nc.gpsimd.partition_broadcast(recb[:], recd[:])

                    def mults(p=p, otu=otu, recb=recb, qsl=qsl):
                        with nc.allow_low_precision("bf16 store"):
                            for hh in range(2):
                                nc.vector.tensor_mul(
                                    ot_t[p][hh * D : (hh + 1) * D, qsl],
                                    otu[hh][0:D, :],
                                    recb[:, hh, :],
                                )
                    if no_defer:
                        mults()
                        if p == 1:
                            for tt in range(4 * cq, 4 * cq + 4):
                                filler.append(
                                    ("out", lambda tt=tt: emit_outproj_tile(tt)))
                    else:
                        pending_mults.append(mults)
                # next chunk's score matmuls read qt/kt of chunk cq+1:
                # force-drain any proj leftovers before emitting them
                drain(kinds=("proj",), limit=99)

            # ---- tail ----
            for fn in pending_mults:
                fn()
            pending_mults.clear()
            for tt in range(4 * (NCHUNK - 1), 4 * NCHUNK):
                filler.append(("out", lambda tt=tt: emit_outproj_tile(tt)))
            drain(limit=99)

    nc.finalize()
    return nc


_NC_CACHE = None


def get_nc():
    global _NC_CACHE
    if _NC_CACHE is None:
        _NC_CACHE = build_nc()
    return _NC_CACHE


def make_in_maps(x, Wq, Wk, Wv, Wo):
    scale = 1.0 / np.sqrt(np.float32(C))
    bf = ml_dtypes.bfloat16
    in_maps = []
    for core in range(8):
        b, hg = core // 4, core % 4
        hsl = slice(hg * NHPC, (hg + 1) * NHPC)
        xT = np.ascontiguousarray(x[b].T.astype(bf))
        wq = (Wq[hsl] * scale).transpose(1, 0, 2).reshape(C, DH)
        wk = Wk[hsl].transpose(1, 0, 2).reshape(C, DH)
        wv = Wv[hsl].transpose(1, 0, 2).reshape(C, DH)
        wqkv = np.ascontiguousarray(
            np.concatenate([wq, wk, wv], axis=1).astype(bf))
        wot = np.ascontiguousarray(Wo[:, hg * DH : (hg + 1) * DH].T.astype(bf))
        in_maps.append({
            "xT": xT,
            "Wqkv": wqkv,
            "WoT": wot,
        })
    return in_maps


def gather(results, bo):
    out = np.zeros((B, T, C), dtype=np.float32)
    for core in range(8):
        out[core // 4] += results[core]["Y"]
    out += bo.astype(np.float32)
    return out


def kernel(x, Wq, Wk, Wv, Wo, bo, **run_kwargs):
    x = np.asarray(x, dtype=np.float32)
    Wq = np.asarray(Wq, dtype=np.float32)
    Wk = np.asarray(Wk, dtype=np.float32)
    Wv = np.asarray(Wv, dtype=np.float32)
    Wo = np.asarray(Wo, dtype=np.float32)
    bo = np.asarray(bo, dtype=np.float32)
    nc = get_nc()
    in_maps = make_in_maps(x, Wq, Wk, Wv, Wo)
    res = run_bass_kernel_spmd(nc, in_maps, core_ids=list(range(8)), **run_kwargs)
    out = gather(res.results, bo)
    if run_kwargs:
        return out, res
    return out
